# revision 36
# baseline (speedup 1.0000x reference)
"""Trainium2 Bass kernel for nn_Encoder_70781061038947.

Math: row b's output depends on x[b, :] only through its 16 sign bits
(root k has radius R if x[b,k] > 0 else 1/R, phase shuffle_vector[k]).
P_b(t) = prod_k (t - z_k) is monic of degree 16, so its 17 coefficients are
determined by the 16 values P_b(t_m) at the 16th roots of unity t_m plus
c_0 = 1.  Split the 16 bits into four 4-bit groups; per group precompute a
16-entry table of (log|E_g(t_m)|, arg E_g(t_m)) on the host (O(1) work).

Device pipeline per core (pure data parallel over B, 32768 rows/core):
  sign bits -> one-hot match counts (PE matmul, 64-row table, two chunks
  stacked in PSUM partition halves) -> one-hot (Act relu + DVE is_equal)
  -> gather log-mag/phase sums (PE matmul vs fp16 table, K=128 sums the 4
  groups in PSUM) -> E = exp(L) (Act) ... phase range-reduce mod 2pi
  (Pool, int32 round trick) -> sin / half-angle cos (Act) -> P = E*(c, s)
  (DVE/Pool) -> transpose 4 tiles at a time (PE) -> 16-point inverse DFT
  via block-diagonal(W2 x4) fp16 matmul (PE) -> q (banded, fp16) -> HBM.

Host finishes with O(B) numpy: c_16 = q_0 - 1, c_d = q_{16-d}, Parseval
norm l2^2 = 1 + |q_0 - 1|^2 + sum_{e>=1} |q_e|^2, scale by sqrt(17)/l2.
Two activation-table phases (exp set, then trig set) avoid ACT_TABLE_LOAD
thrash.
"""

import numpy as np
import ml_dtypes

import concourse.bacc as bacc
import concourse.bass as bass
import concourse.mybir as mybir
import concourse.bass_utils as bass_utils
import concourse.tile as tile

B = 262144
K = 16
M = 16                       # eval points: 16th roots of unity
NCORES = 8
RPC = B // NCORES            # 32768 rows per core
P = 128
NBLK = 8                     # row blocks per core (4096 rows each)
BLKC = RPC // NBLK           # 4096 cols per block
NG = 4                       # bit groups
GS = 4                       # bits per group
TROWS = NG * (1 << GS)       # 64 table rows
TW = 2 * M                   # 32 table cols: L0..15 | A0..15

f32 = mybir.dt.float32
f16 = mybir.dt.float16
bf16 = mybir.dt.bfloat16
i32 = mybir.dt.int32
AF = mybir.ActivationFunctionType
OP = mybir.AluOpType

_cached = None


def _tables(shuffle_vector: np.ndarray):
    sv = np.asarray(shuffle_vector, dtype=np.float64)
    R = np.sqrt(1.0 + np.sin(np.pi / K))
    t = np.exp(2j * np.pi * np.arange(M) / M)
    fp16 = np.float16

    # per-group log-mag/phase tables; table row r = 16*g + nu
    tbl = np.zeros((TROWS, TW), np.float64)
    w3 = np.zeros((K, TROWS), np.float64)      # {0,1}-sign match weights
    n1 = np.zeros(TROWS, np.float64)
    for g in range(NG):
        for nu in range(1 << GS):
            r = 16 * g + nu
            E = np.ones(M, np.complex128)
            for j in range(GS):
                b = (nu >> j) & 1
                zk = (R if b else 1.0 / R) * np.exp(1j * sv[4 * g + j])
                E = E * (t - zk)
                w3[4 * g + j, r] = 2.0 * b - 1.0
            # -1 per group keeps exp(sum L) < 2600, inside fp16 range;
            # the uniform e^4 factor is restored on the host.
            tbl[r, 0:M] = np.log(np.abs(E)) - 1.0
            # phases stored in cycles (units of 2pi): range reduction is
            # then a round() via magic-bias adds, and Sin's scale converts
            tbl[r, M:TW] = np.angle(E) / (2 * np.pi)
            n1[r] = bin(nu).count("1")

    # w3stack [128, 4*128]: K=128 match weights, all matmuls at PE tile
    # (0,0) — mixing tile positions between matmuls faults the hardware.
    # Variant a (pair = blocks a, a+4): out cols 0-63 = block a's table
    # (w3 on rows 16a..16a+16), cols 64-127 = block a+4's (rows 64+16a..).
    w3stack = np.zeros((P, 4 * P), np.float64)
    for am in range(4):
        w3stack[16 * am:16 * am + 16, P * am:P * am + TROWS] = w3
        w3stack[64 + 16 * am:64 + 16 * am + 16,
                P * am + TROWS:P * (am + 1)] = w3
    # +-1 sign convention: match count = sum w~_j s_j = 4 iff all bits match
    biasv = np.full((P, 1), -3.0, np.float32)
    sizev = np.full((P, 1), 4.0, np.float32)

    # zero-padded K=128 gather tables: cols 0:32 for partition-half A
    # (rows 0-63 live), cols 32:64 for half B (rows 64-127 live)
    tbl2 = np.zeros((P, 2 * TW), np.float64)
    tbl2[0:TROWS, 0:TW] = tbl
    tbl2[TROWS:2 * TROWS, TW:2 * TW] = tbl

    # 16-pt inverse DFT in real form: in-comp (re0..15, im0..15) ->
    # out-comp (2e: Re q_e, 2e+1: Im q_e), q_e = (1/16) sum_m Q_m w^{-me}
    W2 = np.zeros((TW, TW), np.float64)
    for m in range(M):
        for e in range(M):
            w = np.exp(-2j * np.pi * m * e / M) / M
            W2[m, 2 * e] = w.real
            W2[m, 2 * e + 1] = w.imag
            W2[M + m, 2 * e] = -w.imag
            W2[M + m, 2 * e + 1] = w.real
    bdw2 = np.zeros((P, P), np.float64)
    for j in range(4):
        bdw2[TW * j:TW * (j + 1), TW * j:TW * (j + 1)] = W2

    return {
        "w3stack": w3stack.astype(fp16),
        "biasv": biasv,
        "sizev": sizev,
        "tbl2": tbl2.astype(fp16),
        "bdw2": bdw2.astype(fp16),
        "ident": np.eye(P, dtype=fp16),
        "magp": np.full((P, 1), 1.5 * 2 ** 23, np.float32),
        "magn": np.full((P, 1), -1.5 * 2 ** 23, np.float32),
    }


def _build_module(stage: int = 99):
    nc = bacc.Bacc("TRN2", target_bir_lowering=False, debug=False)
    x_d = nc.dram_tensor("xT8", [P, RPC * K // P], bf16, kind="ExternalInput")
    assert RPC * K // P == 4096
    w3_d = nc.dram_tensor("w3stack", [P, 4 * P], f16, kind="ExternalInput")
    biasv_d = nc.dram_tensor("biasv", [P, 1], f32, kind="ExternalInput")
    sizev_d = nc.dram_tensor("sizev", [P, 1], f32, kind="ExternalInput")
    tbl_d = nc.dram_tensor("tbl2", [P, 2 * TW], f16, kind="ExternalInput")
    bdw2_d = nc.dram_tensor("bdw2", [P, P], f16, kind="ExternalInput")
    ident_d = nc.dram_tensor("ident", [P, P], f16, kind="ExternalInput")
    magp_d = nc.dram_tensor("magp", [P, 1], f32, kind="ExternalInput")
    magn_d = nc.dram_tensor("magn", [P, 1], f32, kind="ExternalInput")
    q_d = nc.dram_tensor("q", [P, 8192], f16, kind="ExternalOutput")

    XCOLS = 4096             # xT8 free size
    NGRP = 8                 # 2-pair groups, 4096 rows each
    GW = 32 * TW             # 1024: vr/vc cols per group (32 tiles x 32)
    INV2PI = float(1.0 / (2 * np.pi))
    TWOPI = float(2 * np.pi)

    with tile.TileContext(nc) as tc:
        with (
            tc.tile_pool(name="const", bufs=1) as cp,
            tc.tile_pool(name="sb", bufs=3) as sp,
            tc.tile_pool(name="ps", bufs=1, space="PSUM") as pp,
        ):
            w3s = cp.tile([P, 4 * P], f16)
            nc.sync.dma_start(out=w3s[:], in_=w3_d.ap())
            biasv = cp.tile([P, 1], f32)
            nc.sync.dma_start(out=biasv[:], in_=biasv_d.ap())
            sizev = cp.tile([P, 1], f32)
            nc.sync.dma_start(out=sizev[:], in_=sizev_d.ap())
            tbl = cp.tile([P, 2 * TW], f16)
            nc.sync.dma_start(out=tbl[:], in_=tbl_d.ap())
            bdw2 = cp.tile([P, P], f16)
            nc.sync.dma_start(out=bdw2[:], in_=bdw2_d.ap())
            ident = cp.tile([P, P], f16)
            nc.sync.dma_start(out=ident[:], in_=ident_d.ap())
            magp = cp.tile([P, 1], f32)
            nc.sync.dma_start(out=magp[:], in_=magp_d.ap())
            magn = cp.tile([P, 1], f32)
            nc.sync.dma_start(out=magn[:], in_=magn_d.ap())

            xT8 = cp.tile([P, XCOLS], bf16)
            nc.sync.dma_start(out=xT8[:], in_=x_d.ap())

            # persistent across passes
            s = cp.tile([P, XCOLS], f16, name="s")
            E_all = cp.tile([P, NGRP * 512], f16, name="E_all")
            A_all = cp.tile([P, NGRP * 512], f32, name="A_all")

            # signs as +-1: one Act op (Pool tensor_scalar has a ~7us Q7
            # launch overhead per instruction -- keep Pool off the hot path)
            nc.scalar.activation(out=s[:], in_=xT8[:], func=AF.Sign)

            # ---------------- PASS 1: match, one-hot, gather, exp ----------
            for G in range(NGRP):
                a, h = G // 2, G % 2      # pair = (block a, block a+4)
                for half in range(2):     # chunk pair within group
                    colbase = 2048 * h + 1024 * half
                    vr = pp.tile([P, GW // 2], f32, tag="vr", bufs=3)
                    wsl = w3s[:, P * a: P * (a + 1)]
                    for ch in range(2):   # 512-col halves keep mt in one bank
                        cb2 = colbase + 512 * ch
                        mt = pp.tile([P, 512], f32, tag="mt", bufs=2)
                        nc.tensor.matmul(
                            out=mt[:], lhsT=wsl,
                            rhs=s[:, cb2:cb2 + 512], start=True, stop=True)

                        ohh = sp.tile([P, 512], f16, tag="ohh")
                        nc.scalar.activation(
                            out=ohh[:, 0:192], in_=mt[:, 0:192], func=AF.Relu,
                            bias=biasv[:], scale=1.0)
                        nc.vector.tensor_tensor(
                            out=ohh[:, 192:512], in0=mt[:, 192:512],
                            in1=sizev[:].to_broadcast([P, 320]), op=OP.is_equal)

                        # K=128 gathers at tile (0,0); zero-padded table
                        # halves select the partition half
                        for bb in range(2):
                            for t in range(4):
                                tau = 8 * bb + 4 * ch + t
                                nc.tensor.matmul(
                                    out=vr[:, TW * tau: TW * (tau + 1)],
                                    lhsT=ohh[:, 128 * t:128 * (t + 1)],
                                    rhs=tbl[:, TW * bb: TW * (bb + 1)],
                                    start=True, stop=True)

                    p2 = 2 * G + half
                    vrv = vr[:].rearrange("p (t e) -> p t e", e=TW)
                    nc.scalar.activation(
                        out=E_all[:, 256 * p2: 256 * (p2 + 1)].rearrange(
                            "p (t e) -> p t e", e=M),
                        in_=vrv[:, :, 0:M], func=AF.Exp)
                    nc.scalar.activation(
                        out=A_all[:, 256 * p2: 256 * (p2 + 1)].rearrange(
                            "p (t e) -> p t e", e=M),
                        in_=vrv[:, :, M:TW], func=AF.Copy)

            if stage == 1:
                # debug: dump E_all/A_all via q (reinterpret cols)
                eq = sp.tile([P, 512], f16, tag="eqd")
                for G in range(NGRP):
                    nc.vector.tensor_copy(out=eq[:], in_=E_all[:, 512 * G:512 * (G + 1)])
                    nc.scalar.dma_start(
                        out=q_d.ap()[:, 1024 * G:1024 * G + 512], in_=eq[:])
                    eq2 = sp.tile([P, 512], f16, tag="eqd2")
                    nc.vector.tensor_copy(out=eq2[:], in_=A_all[:, 512 * G:512 * (G + 1)])
                    nc.scalar.dma_start(
                        out=q_d.ap()[:, 1024 * G + 512:1024 * (G + 1)], in_=eq2[:])

            # ---------------- PASS 2: trig, assemble, transpose, iDFT ------
            for G in range(NGRP if stage >= 2 else 0):
                Ag = A_all[:, 512 * G: 512 * (G + 1)]
                Eg = E_all[:, 512 * G: 512 * (G + 1)].rearrange(
                    "p (t e) -> p t e", e=M)

                # A is in cycles; k = round(A) via the fp32 magic-bias trick
                # (RTNE in any IEEE ALU, identical in CoreSim and hardware)
                t1 = sp.tile([P, 512], f32, tag="t1")
                nc.vector.tensor_scalar(
                    out=t1[:], in0=Ag, scalar1=float(1.5 * 2 ** 23),
                    scalar2=None, op0=OP.add)
                kk = sp.tile([P, 512], f32, tag="kk")
                nc.scalar.activation(out=kk[:], in_=t1[:], func=AF.Copy,
                                     bias=float(-1.5 * 2 ** 23), scale=1.0)
                ar = sp.tile([P, 512], f32, tag="ar")
                nc.gpsimd.tensor_tensor(out=ar[:], in0=Ag, in1=kk[:], op=OP.subtract)

                sA = sp.tile([P, 512], f16, tag="sA")
                nc.scalar.activation(out=sA[:], in_=ar[:], func=AF.Sin,
                                     scale=float(2 * np.pi * (1.0 - 1e-6)))
                hA = sp.tile([P, 512], f16, tag="hA")
                nc.scalar.activation(out=hA[:], in_=ar[:], func=AF.Sin,
                                     scale=float(np.pi * (1.0 - 1e-6)))
                m2 = sp.tile([P, 512], f16, tag="m2")
                nc.vector.tensor_tensor(out=m2[:], in0=hA[:], in1=hA[:], op=OP.mult)
                cA = sp.tile([P, 512], f16, tag="cA")
                nc.vector.tensor_scalar(
                    out=cA[:], in0=m2[:], scalar1=-2.0, scalar2=1.0,
                    op0=OP.mult, op1=OP.add)

                vc = sp.tile([P, GW], f16, tag="vc")
                vcv = vc[:].rearrange("p (t e) -> p t e", e=TW)
                sAv = sA[:].rearrange("p (t e) -> p t e", e=M)
                cAv = cA[:].rearrange("p (t e) -> p t e", e=M)
                nc.vector.tensor_tensor(out=vcv[:, :, 0:M], in0=Eg, in1=cAv, op=OP.mult)
                nc.gpsimd.tensor_tensor(out=vcv[:, :, M:TW], in0=Eg, in1=sAv, op=OP.mult)

                if stage == 2:
                    nc.scalar.dma_start(
                        out=q_d.ap()[:, 1024 * G:1024 * (G + 1)], in_=vc[:])
                    continue

                for half in range(2):
                    vcT = pp.tile([P, 512], f16, tag="vcT", bufs=2)
                    for t4 in range(4):
                        nc.tensor.transpose(
                            out=vcT[:, 128 * t4:128 * (t4 + 1)],
                            in_=vc[:, 512 * half + 128 * t4: 512 * half + 128 * (t4 + 1)],
                            identity=ident[:])
                    vcTs = sp.tile([P, 512], f16, tag="vcTs")
                    nc.vector.tensor_copy(out=vcTs[:], in_=vcT[:])
                    if stage == 3:
                        nc.scalar.dma_start(
                            out=q_d.ap()[:, 1024 * G + 512 * half:
                                         1024 * G + 512 * (half + 1)],
                            in_=vcTs[:])
                        continue
                    qT = pp.tile([P, 512], f32, tag="mt", bufs=2)
                    nc.tensor.matmul(out=qT[:], lhsT=bdw2[:], rhs=vcTs[:],
                                     start=True, stop=True)
                    qs = sp.tile([P, 512], f16, tag="qs")
                    if half == 0:
                        nc.scalar.activation(out=qs[:], in_=qT[:], func=AF.Copy)
                    else:
                        nc.vector.tensor_copy(out=qs[:], in_=qT[:])
                    nc.sync.dma_start(
                        out=q_d.ap()[:, 1024 * G + 512 * half: 1024 * G + 512 * (half + 1)],
                        in_=qs[:])

    nc.compile()
    return nc


def _prep_inputs(x: np.ndarray, shuffle_vector: np.ndarray):
    x_bf = np.asarray(x).astype(ml_dtypes.bfloat16)
    tabs = _tables(shuffle_vector)
    in_maps = []
    for n in range(NCORES):
        xc = x_bf[n * RPC:(n + 1) * RPC]                    # [32768, 16]
        xT8 = np.ascontiguousarray(
            xc.reshape(NBLK, BLKC, K).transpose(0, 2, 1).reshape(P, BLKC))
        in_maps.append({"xT8": xT8, **tabs})
    return in_maps


def _decode(q_cores: list, shuffle_vector: np.ndarray) -> np.ndarray:
    # q_d [128, 8192] fp16 per core; partition 32j+dc, col 128*tg + l
    # tile tau = 4*(tg % 8) ... see mapping below
    out = np.empty((B, M), np.complex128)
    row_of = np.empty((4, 64, 128), np.int64)    # [j, tg, l] -> row in core
    for tg in range(64):
        G, t4 = tg // 8, tg % 8
        a, h = G // 2, G % 2
        for j in range(4):
            tau = 4 * t4 + j
            cc, t = tau // 8, tau % 8
            blk = a + 4 * (cc % 2)
            colib = 2048 * h + 1024 * (cc // 2) + 128 * t
            row_of[j, tg, :] = 4096 * blk + colib + np.arange(128)
    for n in range(NCORES):
        q = np.asarray(q_cores[n], np.float64).reshape(4, TW, 64, P)  # j, dc, tg, l
        qc = q[:, 0::2] + 1j * q[:, 1::2]                             # j, e, tg, l
        rows = np.empty((RPC, M), np.complex128)
        ro = row_of.reshape(4, 64 * 128)
        for j in range(4):
            rows[ro[j]] = qc[j].reshape(M, 64 * 128).T
        out[n * RPC:(n + 1) * RPC] = rows
    # coefficients: c_0 = 1, c_16 = q_0 - 1, c_d = q_{16-d} for d=1..15
    out *= np.exp(4.0)              # undo the per-group -1 log-mag offset
    q0 = out[:, 0]
    coeffs = np.empty((B, K + 1), np.complex128)
    coeffs[:, 0] = 1.0
    coeffs[:, 16] = q0 - 1.0
    coeffs[:, 1:16] = out[:, 1:][:, ::-1]
    l2 = np.sqrt(np.sum(np.abs(coeffs) ** 2, axis=-1, keepdims=True))
    return coeffs / l2 * np.sqrt(K + 1)


def kernel(x: np.ndarray, shuffle_vector: np.ndarray) -> np.ndarray:
    global _cached
    x = np.asarray(x)
    assert x.shape == (B, K), x.shape
    if _cached is None:
        _cached = _build_module()
    in_maps = _prep_inputs(x, shuffle_vector)
    res = bass_utils.run_bass_kernel_spmd(
        _cached, in_maps, core_ids=list(range(NCORES)))
    return _decode([res.results[n]["q"] for n in range(NCORES)], shuffle_vector)


# revision 37
# speedup vs baseline: 1.1415x; 1.1415x over previous
"""Trainium2 Bass kernel for nn_Encoder_70781061038947.

Math: row b's output depends on x[b, :] only through its 16 sign bits
(root k has radius R if x[b,k] > 0 else 1/R, phase shuffle_vector[k]).
P_b(t) = prod_k (t - z_k) is monic of degree 16, so its 17 coefficients are
determined by the 16 values P_b(t_m) at the 16th roots of unity t_m plus
c_0 = 1.  Split the 16 bits into four 4-bit groups; per group precompute a
16-entry table of (log|E_g(t_m)|, arg E_g(t_m)) on the host (O(1) work).

Device pipeline per core (pure data parallel over B, 32768 rows/core):
  sign bits -> one-hot match counts (PE matmul, 64-row table, two chunks
  stacked in PSUM partition halves) -> one-hot (Act relu + DVE is_equal)
  -> gather log-mag/phase sums (PE matmul vs fp16 table, K=128 sums the 4
  groups in PSUM) -> E = exp(L) (Act) ... phase range-reduce mod 2pi
  (Pool, int32 round trick) -> sin / half-angle cos (Act) -> P = E*(c, s)
  (DVE/Pool) -> transpose 4 tiles at a time (PE) -> 16-point inverse DFT
  via block-diagonal(W2 x4) fp16 matmul (PE) -> q (banded, fp16) -> HBM.

Host finishes with O(B) numpy: c_16 = q_0 - 1, c_d = q_{16-d}, Parseval
norm l2^2 = 1 + |q_0 - 1|^2 + sum_{e>=1} |q_e|^2, scale by sqrt(17)/l2.
Two activation-table phases (exp set, then trig set) avoid ACT_TABLE_LOAD
thrash.
"""

import numpy as np
import ml_dtypes

import concourse.bacc as bacc
import concourse.bass as bass
import concourse.mybir as mybir
import concourse.bass_utils as bass_utils
import concourse.tile as tile

B = 262144
K = 16
M = 16                       # eval points: 16th roots of unity
NCORES = 8
RPC = B // NCORES            # 32768 rows per core
P = 128
NBLK = 8                     # row blocks per core (4096 rows each)
BLKC = RPC // NBLK           # 4096 cols per block
NG = 4                       # bit groups
GS = 4                       # bits per group
TROWS = NG * (1 << GS)       # 64 table rows
TW = 2 * M                   # 32 table cols: L0..15 | A0..15

f32 = mybir.dt.float32
f16 = mybir.dt.float16
bf16 = mybir.dt.bfloat16
i32 = mybir.dt.int32
AF = mybir.ActivationFunctionType
OP = mybir.AluOpType

_cached = None


def _tables(shuffle_vector: np.ndarray):
    sv = np.asarray(shuffle_vector, dtype=np.float64)
    R = np.sqrt(1.0 + np.sin(np.pi / K))
    t = np.exp(2j * np.pi * np.arange(M) / M)
    fp16 = np.float16

    # per-group log-mag/phase tables; table row r = 16*g + nu
    tbl = np.zeros((TROWS, TW), np.float64)
    w3 = np.zeros((K, TROWS), np.float64)      # {0,1}-sign match weights
    n1 = np.zeros(TROWS, np.float64)
    for g in range(NG):
        for nu in range(1 << GS):
            r = 16 * g + nu
            E = np.ones(M, np.complex128)
            for j in range(GS):
                b = (nu >> j) & 1
                zk = (R if b else 1.0 / R) * np.exp(1j * sv[4 * g + j])
                E = E * (t - zk)
                w3[4 * g + j, r] = 2.0 * b - 1.0
            # -1 per group keeps exp(sum L) < 2600, inside fp16 range;
            # the uniform e^4 factor is restored on the host.
            tbl[r, 0:M] = np.log(np.abs(E)) - 1.0
            # phases stored in cycles (units of 2pi): range reduction is
            # then a round() via magic-bias adds, and Sin's scale converts
            tbl[r, M:TW] = np.angle(E) / (2 * np.pi)
            n1[r] = bin(nu).count("1")

    # w3stack [128, 4*128]: K=128 match weights, all matmuls at PE tile
    # (0,0) — mixing tile positions between matmuls faults the hardware.
    # Variant a (pair = blocks a, a+4): out cols 0-63 = block a's table
    # (w3 on rows 16a..16a+16), cols 64-127 = block a+4's (rows 64+16a..).
    w3stack = np.zeros((P, 4 * P), np.float64)
    for am in range(4):
        w3stack[16 * am:16 * am + 16, P * am:P * am + TROWS] = w3
        w3stack[64 + 16 * am:64 + 16 * am + 16,
                P * am + TROWS:P * (am + 1)] = w3
    # +-1 sign convention: match count = sum w~_j s_j = 4 iff all bits match
    biasv = np.full((P, 1), -3.0, np.float32)
    sizev = np.full((P, 1), 4.0, np.float32)

    # zero-padded K=128 gather tables: cols 0:32 for partition-half A
    # (rows 0-63 live), cols 32:64 for half B (rows 64-127 live)
    tbl2 = np.zeros((P, 2 * TW), np.float64)
    tbl2[0:TROWS, 0:TW] = tbl
    tbl2[TROWS:2 * TROWS, TW:2 * TW] = tbl

    # 16-pt inverse DFT in real form: in-comp (re0..15, im0..15) ->
    # out-comp (2e: Re q_e, 2e+1: Im q_e), q_e = (1/16) sum_m Q_m w^{-me}
    W2 = np.zeros((TW, TW), np.float64)
    for m in range(M):
        for e in range(M):
            w = np.exp(-2j * np.pi * m * e / M) / M
            W2[m, 2 * e] = w.real
            W2[m, 2 * e + 1] = w.imag
            W2[M + m, 2 * e] = -w.imag
            W2[M + m, 2 * e + 1] = w.real
    bdw2 = np.zeros((P, P), np.float64)
    for j in range(4):
        bdw2[TW * j:TW * (j + 1), TW * j:TW * (j + 1)] = W2

    return {
        "w3stack": w3stack.astype(fp16),
        "biasv": biasv,
        "sizev": sizev,
        "tbl2": tbl2.astype(fp16),
        "bdw2": bdw2.astype(fp16),
        "ident": np.eye(P, dtype=fp16),
        "magp": np.full((P, 1), 1.5 * 2 ** 23, np.float32),
        "magn": np.full((P, 1), -1.5 * 2 ** 23, np.float32),
    }


def _build_module(stage: int = 99):
    nc = bacc.Bacc("TRN2", target_bir_lowering=False, debug=False)
    x_d = nc.dram_tensor("xT8", [P, RPC * K // P], bf16, kind="ExternalInput")
    assert RPC * K // P == 4096
    w3_d = nc.dram_tensor("w3stack", [P, 4 * P], f16, kind="ExternalInput")
    biasv_d = nc.dram_tensor("biasv", [P, 1], f32, kind="ExternalInput")
    sizev_d = nc.dram_tensor("sizev", [P, 1], f32, kind="ExternalInput")
    tbl_d = nc.dram_tensor("tbl2", [P, 2 * TW], f16, kind="ExternalInput")
    bdw2_d = nc.dram_tensor("bdw2", [P, P], f16, kind="ExternalInput")
    ident_d = nc.dram_tensor("ident", [P, P], f16, kind="ExternalInput")
    magp_d = nc.dram_tensor("magp", [P, 1], f32, kind="ExternalInput")
    magn_d = nc.dram_tensor("magn", [P, 1], f32, kind="ExternalInput")
    q_d = nc.dram_tensor("q", [P, 8192], f16, kind="ExternalOutput")

    XCOLS = 4096             # xT8 free size
    NGRP = 8                 # 2-pair groups, 4096 rows each
    GW = 32 * TW             # 1024: vr/vc cols per group (32 tiles x 32)
    INV2PI = float(1.0 / (2 * np.pi))
    TWOPI = float(2 * np.pi)

    with tile.TileContext(nc) as tc:
        with (
            tc.tile_pool(name="const", bufs=1) as cp,
            tc.tile_pool(name="sb", bufs=3) as sp,
            tc.tile_pool(name="ps", bufs=1, space="PSUM") as pp,
        ):
            w3s = cp.tile([P, 4 * P], f16)
            nc.sync.dma_start(out=w3s[:], in_=w3_d.ap())
            biasv = cp.tile([P, 1], f32)
            nc.sync.dma_start(out=biasv[:], in_=biasv_d.ap())
            sizev = cp.tile([P, 1], f32)
            nc.sync.dma_start(out=sizev[:], in_=sizev_d.ap())
            tbl = cp.tile([P, 2 * TW], f16)
            nc.sync.dma_start(out=tbl[:], in_=tbl_d.ap())
            bdw2 = cp.tile([P, P], f16)
            nc.sync.dma_start(out=bdw2[:], in_=bdw2_d.ap())
            ident = cp.tile([P, P], f16)
            nc.sync.dma_start(out=ident[:], in_=ident_d.ap())
            magp = cp.tile([P, 1], f32)
            nc.sync.dma_start(out=magp[:], in_=magp_d.ap())
            magn = cp.tile([P, 1], f32)
            nc.sync.dma_start(out=magn[:], in_=magn_d.ap())

            xT8 = cp.tile([P, XCOLS], bf16)
            nc.sync.dma_start(out=xT8[:], in_=x_d.ap())

            # persistent across passes
            s = cp.tile([P, XCOLS], f16, name="s")
            E_all = cp.tile([P, NGRP * 512], f16, name="E_all")
            A_all = cp.tile([P, NGRP * 512], f32, name="A_all")

            # signs as +-1: one Act op (Pool tensor_scalar has a ~7us Q7
            # launch overhead per instruction -- keep Pool off the hot path)
            nc.scalar.activation(out=s[:], in_=xT8[:], func=AF.Sign)

            # ---------------- PASS 1: match, one-hot, gather, exp ----------
            for G in range(NGRP):
                a, h = G // 2, G % 2      # pair = (block a, block a+4)
                for half in range(2):     # chunk pair within group
                    colbase = 2048 * h + 1024 * half
                    vr = pp.tile([P, GW // 2], f32, tag="vr", bufs=3)
                    wsl = w3s[:, P * a: P * (a + 1)]
                    for ch in range(2):   # 512-col halves keep mt in one bank
                        cb2 = colbase + 512 * ch
                        mt = pp.tile([P, 512], f32, tag="mt", bufs=2)
                        nc.tensor.matmul(
                            out=mt[:], lhsT=wsl,
                            rhs=s[:, cb2:cb2 + 512], start=True, stop=True)

                        ohh = sp.tile([P, 512], f16, tag="ohh")
                        nc.scalar.activation(
                            out=ohh[:, 0:192], in_=mt[:, 0:192], func=AF.Relu,
                            bias=biasv[:], scale=1.0)
                        nc.vector.tensor_tensor(
                            out=ohh[:, 192:512], in0=mt[:, 192:512],
                            in1=sizev[:].to_broadcast([P, 320]), op=OP.is_equal)

                        # K=128 gathers at tile (0,0); zero-padded table
                        # halves select the partition half
                        for bb in range(2):
                            for t in range(4):
                                tau = 8 * bb + 4 * ch + t
                                nc.tensor.matmul(
                                    out=vr[:, TW * tau: TW * (tau + 1)],
                                    lhsT=ohh[:, 128 * t:128 * (t + 1)],
                                    rhs=tbl[:, TW * bb: TW * (bb + 1)],
                                    start=True, stop=True)

                    p2 = 2 * G + half
                    vrv = vr[:].rearrange("p (t e) -> p t e", e=TW)
                    nc.scalar.activation(
                        out=E_all[:, 256 * p2: 256 * (p2 + 1)].rearrange(
                            "p (t e) -> p t e", e=M),
                        in_=vrv[:, :, 0:M], func=AF.Exp)
                    nc.scalar.activation(
                        out=A_all[:, 256 * p2: 256 * (p2 + 1)].rearrange(
                            "p (t e) -> p t e", e=M),
                        in_=vrv[:, :, M:TW], func=AF.Copy)

            if stage == 1:
                # debug: dump E_all/A_all via q (reinterpret cols)
                eq = sp.tile([P, 512], f16, tag="eqd")
                for G in range(NGRP):
                    nc.vector.tensor_copy(out=eq[:], in_=E_all[:, 512 * G:512 * (G + 1)])
                    nc.scalar.dma_start(
                        out=q_d.ap()[:, 1024 * G:1024 * G + 512], in_=eq[:])
                    eq2 = sp.tile([P, 512], f16, tag="eqd2")
                    nc.vector.tensor_copy(out=eq2[:], in_=A_all[:, 512 * G:512 * (G + 1)])
                    nc.scalar.dma_start(
                        out=q_d.ap()[:, 1024 * G + 512:1024 * (G + 1)], in_=eq2[:])

            # scheduler fence: keep every engine's pass-1 ops ahead of its
            # pass-2 ops so Act does all Exps, then all Sins (2 table loads)
            tc.no_sync_barrier()

            # ---------------- PASS 2: trig, assemble, transpose, iDFT ------
            for G in range(NGRP if stage >= 2 else 0):
                Ag = A_all[:, 512 * G: 512 * (G + 1)]
                Eg = E_all[:, 512 * G: 512 * (G + 1)].rearrange(
                    "p (t e) -> p t e", e=M)

                # A is in cycles; k = round(A) via the fp32 magic-bias trick
                # (RTNE in any IEEE ALU, identical in CoreSim and hardware)
                t1 = sp.tile([P, 512], f32, tag="t1")
                nc.vector.tensor_scalar(
                    out=t1[:], in0=Ag, scalar1=float(1.5 * 2 ** 23),
                    scalar2=None, op0=OP.add)
                kk = sp.tile([P, 512], f32, tag="kk")
                nc.scalar.activation(out=kk[:], in_=t1[:], func=AF.Copy,
                                     bias=float(-1.5 * 2 ** 23), scale=1.0)
                ar = sp.tile([P, 512], f32, tag="ar")
                nc.vector.tensor_tensor(out=ar[:], in0=Ag, in1=kk[:], op=OP.subtract)

                sA = sp.tile([P, 512], f16, tag="sA")
                nc.scalar.activation(out=sA[:], in_=ar[:], func=AF.Sin,
                                     scale=float(2 * np.pi * (1.0 - 1e-6)))
                hA = sp.tile([P, 512], f16, tag="hA")
                nc.scalar.activation(out=hA[:], in_=ar[:], func=AF.Sin,
                                     scale=float(np.pi * (1.0 - 1e-6)))
                m2 = sp.tile([P, 512], f16, tag="m2")
                nc.vector.tensor_tensor(out=m2[:], in0=hA[:], in1=hA[:], op=OP.mult)
                cA = sp.tile([P, 512], f16, tag="cA")
                nc.vector.tensor_scalar(
                    out=cA[:], in0=m2[:], scalar1=-2.0, scalar2=1.0,
                    op0=OP.mult, op1=OP.add)

                vc = sp.tile([P, GW], f16, tag="vc")
                vcv = vc[:].rearrange("p (t e) -> p t e", e=TW)
                sAv = sA[:].rearrange("p (t e) -> p t e", e=M)
                cAv = cA[:].rearrange("p (t e) -> p t e", e=M)
                nc.vector.tensor_tensor(out=vcv[:, :, 0:M], in0=Eg, in1=cAv, op=OP.mult)
                nc.gpsimd.tensor_tensor(out=vcv[:, :, M:TW], in0=Eg, in1=sAv, op=OP.mult)

                if stage == 2:
                    nc.scalar.dma_start(
                        out=q_d.ap()[:, 1024 * G:1024 * (G + 1)], in_=vc[:])
                    continue

                for half in range(2):
                    vcT = pp.tile([P, 512], f16, tag="vcT", bufs=2)
                    for t4 in range(4):
                        nc.tensor.transpose(
                            out=vcT[:, 128 * t4:128 * (t4 + 1)],
                            in_=vc[:, 512 * half + 128 * t4: 512 * half + 128 * (t4 + 1)],
                            identity=ident[:])
                    vcTs = sp.tile([P, 512], f16, tag="vcTs")
                    nc.vector.tensor_copy(out=vcTs[:], in_=vcT[:])
                    if stage == 3:
                        nc.scalar.dma_start(
                            out=q_d.ap()[:, 1024 * G + 512 * half:
                                         1024 * G + 512 * (half + 1)],
                            in_=vcTs[:])
                        continue
                    qT = pp.tile([P, 512], f32, tag="mt", bufs=2)
                    nc.tensor.matmul(out=qT[:], lhsT=bdw2[:], rhs=vcTs[:],
                                     start=True, stop=True)
                    qs = sp.tile([P, 512], f16, tag="qs")
                    if half == 0:
                        nc.scalar.activation(out=qs[:], in_=qT[:], func=AF.Copy)
                    else:
                        nc.vector.tensor_copy(out=qs[:], in_=qT[:])
                    nc.sync.dma_start(
                        out=q_d.ap()[:, 1024 * G + 512 * half: 1024 * G + 512 * (half + 1)],
                        in_=qs[:])

    nc.compile()
    return nc


def _prep_inputs(x: np.ndarray, shuffle_vector: np.ndarray):
    x_bf = np.asarray(x).astype(ml_dtypes.bfloat16)
    tabs = _tables(shuffle_vector)
    in_maps = []
    for n in range(NCORES):
        xc = x_bf[n * RPC:(n + 1) * RPC]                    # [32768, 16]
        xT8 = np.ascontiguousarray(
            xc.reshape(NBLK, BLKC, K).transpose(0, 2, 1).reshape(P, BLKC))
        in_maps.append({"xT8": xT8, **tabs})
    return in_maps


def _decode(q_cores: list, shuffle_vector: np.ndarray) -> np.ndarray:
    # q_d [128, 8192] fp16 per core; partition 32j+dc, col 128*tg + l
    # tile tau = 4*(tg % 8) ... see mapping below
    out = np.empty((B, M), np.complex128)
    row_of = np.empty((4, 64, 128), np.int64)    # [j, tg, l] -> row in core
    for tg in range(64):
        G, t4 = tg // 8, tg % 8
        a, h = G // 2, G % 2
        for j in range(4):
            tau = 4 * t4 + j
            cc, t = tau // 8, tau % 8
            blk = a + 4 * (cc % 2)
            colib = 2048 * h + 1024 * (cc // 2) + 128 * t
            row_of[j, tg, :] = 4096 * blk + colib + np.arange(128)
    for n in range(NCORES):
        q = np.asarray(q_cores[n], np.float64).reshape(4, TW, 64, P)  # j, dc, tg, l
        qc = q[:, 0::2] + 1j * q[:, 1::2]                             # j, e, tg, l
        rows = np.empty((RPC, M), np.complex128)
        ro = row_of.reshape(4, 64 * 128)
        for j in range(4):
            rows[ro[j]] = qc[j].reshape(M, 64 * 128).T
        out[n * RPC:(n + 1) * RPC] = rows
    # coefficients: c_0 = 1, c_16 = q_0 - 1, c_d = q_{16-d} for d=1..15
    out *= np.exp(4.0)              # undo the per-group -1 log-mag offset
    q0 = out[:, 0]
    coeffs = np.empty((B, K + 1), np.complex128)
    coeffs[:, 0] = 1.0
    coeffs[:, 16] = q0 - 1.0
    coeffs[:, 1:16] = out[:, 1:][:, ::-1]
    l2 = np.sqrt(np.sum(np.abs(coeffs) ** 2, axis=-1, keepdims=True))
    return coeffs / l2 * np.sqrt(K + 1)


def kernel(x: np.ndarray, shuffle_vector: np.ndarray) -> np.ndarray:
    global _cached
    x = np.asarray(x)
    assert x.shape == (B, K), x.shape
    if _cached is None:
        _cached = _build_module()
    in_maps = _prep_inputs(x, shuffle_vector)
    res = bass_utils.run_bass_kernel_spmd(
        _cached, in_maps, core_ids=list(range(NCORES)))
    return _decode([res.results[n]["q"] for n in range(NCORES)], shuffle_vector)


# revision 38
# speedup vs baseline: 1.1739x; 1.0284x over previous
"""Trainium2 Bass kernel for nn_Encoder_70781061038947.

Math: row b's output depends on x[b, :] only through its 16 sign bits
(root k has radius R if x[b,k] > 0 else 1/R, phase shuffle_vector[k]).
P_b(t) = prod_k (t - z_k) is monic of degree 16, so its 17 coefficients are
determined by the 16 values P_b(t_m) at the 16th roots of unity t_m plus
c_0 = 1.  Split the 16 bits into four 4-bit groups; per group precompute a
16-entry table of (log|E_g(t_m)|, arg E_g(t_m)) on the host (O(1) work).

Device pipeline per core (pure data parallel over B, 32768 rows/core):
  sign bits -> one-hot match counts (PE matmul, 64-row table, two chunks
  stacked in PSUM partition halves) -> one-hot (Act relu + DVE is_equal)
  -> gather log-mag/phase sums (PE matmul vs fp16 table, K=128 sums the 4
  groups in PSUM) -> E = exp(L) (Act) ... phase range-reduce mod 2pi
  (Pool, int32 round trick) -> sin / half-angle cos (Act) -> P = E*(c, s)
  (DVE/Pool) -> transpose 4 tiles at a time (PE) -> 16-point inverse DFT
  via block-diagonal(W2 x4) fp16 matmul (PE) -> q (banded, fp16) -> HBM.

Host finishes with O(B) numpy: c_16 = q_0 - 1, c_d = q_{16-d}, Parseval
norm l2^2 = 1 + |q_0 - 1|^2 + sum_{e>=1} |q_e|^2, scale by sqrt(17)/l2.
Two activation-table phases (exp set, then trig set) avoid ACT_TABLE_LOAD
thrash.
"""

import numpy as np
import ml_dtypes

import concourse.bacc as bacc
import concourse.bass as bass
import concourse.mybir as mybir
import concourse.bass_utils as bass_utils
import concourse.tile as tile

B = 262144
K = 16
M = 16                       # eval points: 16th roots of unity
NCORES = 8
RPC = B // NCORES            # 32768 rows per core
P = 128
NBLK = 8                     # row blocks per core (4096 rows each)
BLKC = RPC // NBLK           # 4096 cols per block
NG = 4                       # bit groups
GS = 4                       # bits per group
TROWS = NG * (1 << GS)       # 64 table rows
TW = 2 * M                   # 32 table cols: L0..15 | A0..15

f32 = mybir.dt.float32
f16 = mybir.dt.float16
bf16 = mybir.dt.bfloat16
i32 = mybir.dt.int32
AF = mybir.ActivationFunctionType
OP = mybir.AluOpType

_cached = None


def _tables(shuffle_vector: np.ndarray):
    sv = np.asarray(shuffle_vector, dtype=np.float64)
    R = np.sqrt(1.0 + np.sin(np.pi / K))
    t = np.exp(2j * np.pi * np.arange(M) / M)
    fp16 = np.float16

    # per-group log-mag/phase tables; table row r = 16*g + nu
    tbl = np.zeros((TROWS, TW), np.float64)
    w3 = np.zeros((K, TROWS), np.float64)      # {0,1}-sign match weights
    n1 = np.zeros(TROWS, np.float64)
    for g in range(NG):
        for nu in range(1 << GS):
            r = 16 * g + nu
            E = np.ones(M, np.complex128)
            for j in range(GS):
                b = (nu >> j) & 1
                zk = (R if b else 1.0 / R) * np.exp(1j * sv[4 * g + j])
                E = E * (t - zk)
                w3[4 * g + j, r] = 2.0 * b - 1.0
            # -1 per group keeps exp(sum L) < 2600, inside fp16 range;
            # the uniform e^4 factor is restored on the host.
            tbl[r, 0:M] = np.log(np.abs(E)) - 1.0
            # phases stored in cycles (units of 2pi): range reduction is
            # then a round() via magic-bias adds, and Sin's scale converts
            tbl[r, M:TW] = np.angle(E) / (2 * np.pi)
            n1[r] = bin(nu).count("1")

    # w3stack [128, 4*128]: K=128 match weights, all matmuls at PE tile
    # (0,0) — mixing tile positions between matmuls faults the hardware.
    # Variant a (pair = blocks a, a+4): out cols 0-63 = block a's table
    # (w3 on rows 16a..16a+16), cols 64-127 = block a+4's (rows 64+16a..).
    w3stack = np.zeros((P, 4 * P), np.float64)
    for am in range(4):
        w3stack[16 * am:16 * am + 16, P * am:P * am + TROWS] = w3
        w3stack[64 + 16 * am:64 + 16 * am + 16,
                P * am + TROWS:P * (am + 1)] = w3
    # +-1 sign convention: match count = sum w~_j s_j = 4 iff all bits match
    biasv = np.full((P, 1), -3.0, np.float32)
    sizev = np.full((P, 1), 4.0, np.float32)

    # zero-padded K=128 gather tables: cols 0:32 for partition-half A
    # (rows 0-63 live), cols 32:64 for half B (rows 64-127 live)
    tbl2 = np.zeros((P, 2 * TW), np.float64)
    tbl2[0:TROWS, 0:TW] = tbl
    tbl2[TROWS:2 * TROWS, TW:2 * TW] = tbl

    # 16-pt inverse DFT in real form: in-comp (re0..15, im0..15) ->
    # out-comp (2e: Re q_e, 2e+1: Im q_e), q_e = (1/16) sum_m Q_m w^{-me}
    W2 = np.zeros((TW, TW), np.float64)
    for m in range(M):
        for e in range(M):
            w = np.exp(-2j * np.pi * m * e / M) / M
            W2[m, 2 * e] = w.real
            W2[m, 2 * e + 1] = w.imag
            W2[M + m, 2 * e] = -w.imag
            W2[M + m, 2 * e + 1] = w.real
    bdw2 = np.zeros((P, P), np.float64)
    for j in range(4):
        bdw2[TW * j:TW * (j + 1), TW * j:TW * (j + 1)] = W2

    return {
        "w3stack": w3stack.astype(fp16),
        "biasv": biasv,
        "sizev": sizev,
        "tbl2": tbl2.astype(fp16),
        "bdw2": bdw2.astype(fp16),
        "ident": np.eye(P, dtype=fp16),
        "magp": np.full((P, 1), 1.5 * 2 ** 23, np.float32),
        "magn": np.full((P, 1), -1.5 * 2 ** 23, np.float32),
    }


def _build_module(stage: int = 99):
    nc = bacc.Bacc("TRN2", target_bir_lowering=False, debug=False)
    x_d = nc.dram_tensor("xT8", [P, RPC * K // P], bf16, kind="ExternalInput")
    assert RPC * K // P == 4096
    w3_d = nc.dram_tensor("w3stack", [P, 4 * P], f16, kind="ExternalInput")
    biasv_d = nc.dram_tensor("biasv", [P, 1], f32, kind="ExternalInput")
    sizev_d = nc.dram_tensor("sizev", [P, 1], f32, kind="ExternalInput")
    tbl_d = nc.dram_tensor("tbl2", [P, 2 * TW], f16, kind="ExternalInput")
    bdw2_d = nc.dram_tensor("bdw2", [P, P], f16, kind="ExternalInput")
    ident_d = nc.dram_tensor("ident", [P, P], f16, kind="ExternalInput")
    magp_d = nc.dram_tensor("magp", [P, 1], f32, kind="ExternalInput")
    magn_d = nc.dram_tensor("magn", [P, 1], f32, kind="ExternalInput")
    q_d = nc.dram_tensor("q", [P, 8192], f16, kind="ExternalOutput")

    XCOLS = 4096             # xT8 free size
    NGRP = 8                 # 2-pair groups, 4096 rows each
    GW = 32 * TW             # 1024: vr/vc cols per group (32 tiles x 32)
    INV2PI = float(1.0 / (2 * np.pi))
    TWOPI = float(2 * np.pi)

    with tile.TileContext(nc) as tc:
        with (
            tc.tile_pool(name="const", bufs=1) as cp,
            tc.tile_pool(name="sb", bufs=3) as sp,
            tc.tile_pool(name="ps", bufs=1, space="PSUM") as pp,
        ):
            w3s = cp.tile([P, 4 * P], f16)
            nc.sync.dma_start(out=w3s[:], in_=w3_d.ap())
            biasv = cp.tile([P, 1], f32)
            nc.sync.dma_start(out=biasv[:], in_=biasv_d.ap())
            sizev = cp.tile([P, 1], f32)
            nc.sync.dma_start(out=sizev[:], in_=sizev_d.ap())
            tbl = cp.tile([P, 2 * TW], f16)
            nc.sync.dma_start(out=tbl[:], in_=tbl_d.ap())
            bdw2 = cp.tile([P, P], f16)
            nc.sync.dma_start(out=bdw2[:], in_=bdw2_d.ap())
            ident = cp.tile([P, P], f16)
            nc.sync.dma_start(out=ident[:], in_=ident_d.ap())
            magp = cp.tile([P, 1], f32)
            nc.sync.dma_start(out=magp[:], in_=magp_d.ap())
            magn = cp.tile([P, 1], f32)
            nc.sync.dma_start(out=magn[:], in_=magn_d.ap())

            xT8 = cp.tile([P, XCOLS], bf16)
            nc.sync.dma_start(out=xT8[:], in_=x_d.ap())

            # persistent across passes
            s = cp.tile([P, XCOLS], f16, name="s")
            E_all = cp.tile([P, NGRP * 512], f16, name="E_all")
            A_all = cp.tile([P, NGRP * 512], f32, name="A_all")

            # signs as +-1: one Act op (Pool tensor_scalar has a ~7us Q7
            # launch overhead per instruction -- keep Pool off the hot path)
            nc.scalar.activation(out=s[:], in_=xT8[:], func=AF.Sign)

            # ---------------- PASS 1: match, one-hot, gather, exp ----------
            for G in range(NGRP):
                a, h = G // 2, G % 2      # pair = (block a, block a+4)
                for half in range(2):     # chunk pair within group
                    colbase = 2048 * h + 1024 * half
                    vr = pp.tile([P, GW // 2], f32, tag="vr", bufs=2)
                    wsl = w3s[:, P * a: P * (a + 1)]
                    mt = pp.tile([P, 1024], f32, tag="mt", bufs=2)
                    for ch in range(2):   # moving-dim cap is 512
                        cb2 = colbase + 512 * ch
                        nc.tensor.matmul(
                            out=mt[:, 512 * ch:512 * (ch + 1)], lhsT=wsl,
                            rhs=s[:, cb2:cb2 + 512], start=True, stop=True)
                    ohh = sp.tile([P, 1024], f16, tag="ohh")
                    nc.scalar.activation(
                        out=ohh[:, 0:384], in_=mt[:, 0:384], func=AF.Relu,
                        bias=biasv[:], scale=1.0)
                    nc.vector.tensor_tensor(
                        out=ohh[:, 384:1024], in0=mt[:, 384:1024],
                        in1=sizev[:].to_broadcast([P, 640]), op=OP.is_equal)

                    # K=128 gathers at tile (0,0); zero-padded table
                    # halves select the partition half
                    for bb in range(2):
                        for ch in range(2):
                            for t in range(4):
                                tau = 8 * bb + 4 * ch + t
                                nc.tensor.matmul(
                                    out=vr[:, TW * tau: TW * (tau + 1)],
                                    lhsT=ohh[:, 512 * ch + 128 * t:
                                             512 * ch + 128 * (t + 1)],
                                    rhs=tbl[:, TW * bb: TW * (bb + 1)],
                                    start=True, stop=True)

                    p2 = 2 * G + half
                    vrv = vr[:].rearrange("p (t e) -> p t e", e=TW)
                    nc.scalar.activation(
                        out=E_all[:, 256 * p2: 256 * (p2 + 1)].rearrange(
                            "p (t e) -> p t e", e=M),
                        in_=vrv[:, :, 0:M], func=AF.Exp)
                    nc.scalar.activation(
                        out=A_all[:, 256 * p2: 256 * (p2 + 1)].rearrange(
                            "p (t e) -> p t e", e=M),
                        in_=vrv[:, :, M:TW], func=AF.Copy)

            if stage == 1:
                # debug: dump E_all/A_all via q (reinterpret cols)
                eq = sp.tile([P, 512], f16, tag="eqd")
                for G in range(NGRP):
                    nc.vector.tensor_copy(out=eq[:], in_=E_all[:, 512 * G:512 * (G + 1)])
                    nc.scalar.dma_start(
                        out=q_d.ap()[:, 1024 * G:1024 * G + 512], in_=eq[:])
                    eq2 = sp.tile([P, 512], f16, tag="eqd2")
                    nc.vector.tensor_copy(out=eq2[:], in_=A_all[:, 512 * G:512 * (G + 1)])
                    nc.scalar.dma_start(
                        out=q_d.ap()[:, 1024 * G + 512:1024 * (G + 1)], in_=eq2[:])

            # scheduler fence: keep every engine's pass-1 ops ahead of its
            # pass-2 ops so Act does all Exps, then all Sins (2 table loads)
            tc.no_sync_barrier()

            # ---------------- PASS 2: trig, assemble, transpose, iDFT ------
            for GG in range(4 if stage >= 2 else 0):   # 2 G-groups at a time
                Ag = A_all[:, 1024 * GG: 1024 * (GG + 1)]
                Eg = E_all[:, 1024 * GG: 1024 * (GG + 1)].rearrange(
                    "p (t e) -> p t e", e=M)

                # A is in cycles; k = round(A) via the fp32 magic-bias trick
                # (RTNE in any IEEE ALU, identical in CoreSim and hardware)
                t1 = sp.tile([P, 1024], f32, tag="t1")
                nc.vector.tensor_scalar(
                    out=t1[:], in0=Ag, scalar1=float(1.5 * 2 ** 23),
                    scalar2=None, op0=OP.add)
                kk = sp.tile([P, 1024], f32, tag="kk")
                nc.scalar.activation(out=kk[:], in_=t1[:], func=AF.Copy,
                                     bias=float(-1.5 * 2 ** 23), scale=1.0)
                ar = sp.tile([P, 1024], f32, tag="ar")
                nc.vector.tensor_tensor(out=ar[:], in0=Ag, in1=kk[:], op=OP.subtract)

                sA = sp.tile([P, 1024], f16, tag="sA")
                nc.scalar.activation(out=sA[:], in_=ar[:], func=AF.Sin,
                                     scale=float(2 * np.pi * (1.0 - 1e-6)))
                hA = sp.tile([P, 1024], f16, tag="hA")
                nc.scalar.activation(out=hA[:], in_=ar[:], func=AF.Sin,
                                     scale=float(np.pi * (1.0 - 1e-6)))
                m2 = sp.tile([P, 1024], f16, tag="m2")
                nc.gpsimd.tensor_tensor(out=m2[:], in0=hA[:], in1=hA[:], op=OP.mult)
                cA = sp.tile([P, 1024], f16, tag="cA")
                nc.vector.tensor_scalar(
                    out=cA[:], in0=m2[:], scalar1=-2.0, scalar2=1.0,
                    op0=OP.mult, op1=OP.add)

                vc = sp.tile([P, 2 * GW], f16, tag="vc")
                vcv = vc[:].rearrange("p (t e) -> p t e", e=TW)
                sAv = sA[:].rearrange("p (t e) -> p t e", e=M)
                cAv = cA[:].rearrange("p (t e) -> p t e", e=M)
                nc.vector.tensor_tensor(out=vcv[:, :, 0:M], in0=Eg, in1=cAv, op=OP.mult)
                nc.gpsimd.tensor_tensor(out=vcv[:, :, M:TW], in0=Eg, in1=sAv, op=OP.mult)

                for half in range(4):
                    vcT = pp.tile([P, 512], f16, tag="vcT", bufs=2)
                    for t4 in range(4):
                        nc.tensor.transpose(
                            out=vcT[:, 128 * t4:128 * (t4 + 1)],
                            in_=vc[:, 512 * half + 128 * t4: 512 * half + 128 * (t4 + 1)],
                            identity=ident[:])
                    vcTs = sp.tile([P, 512], f16, tag="vcTs")
                    nc.vector.tensor_copy(out=vcTs[:], in_=vcT[:])
                    qT = pp.tile([P, 512], f32, tag="vr", bufs=2)
                    nc.tensor.matmul(out=qT[:], lhsT=bdw2[:], rhs=vcTs[:],
                                     start=True, stop=True)
                    qs = sp.tile([P, 512], f16, tag="qs")
                    if half % 2 == 0:
                        nc.scalar.activation(out=qs[:], in_=qT[:], func=AF.Copy)
                    else:
                        nc.vector.tensor_copy(out=qs[:], in_=qT[:])
                    nc.sync.dma_start(
                        out=q_d.ap()[:, 2048 * GG + 512 * half:
                                     2048 * GG + 512 * (half + 1)],
                        in_=qs[:])

    nc.compile()
    return nc


def _prep_inputs(x: np.ndarray, shuffle_vector: np.ndarray):
    x_bf = np.asarray(x).astype(ml_dtypes.bfloat16)
    tabs = _tables(shuffle_vector)
    in_maps = []
    for n in range(NCORES):
        xc = x_bf[n * RPC:(n + 1) * RPC]                    # [32768, 16]
        xT8 = np.ascontiguousarray(
            xc.reshape(NBLK, BLKC, K).transpose(0, 2, 1).reshape(P, BLKC))
        in_maps.append({"xT8": xT8, **tabs})
    return in_maps


def _decode(q_cores: list, shuffle_vector: np.ndarray) -> np.ndarray:
    # q_d [128, 8192] fp16 per core; partition 32j+dc, col 128*tg + l
    # tile tau = 4*(tg % 8) ... see mapping below
    out = np.empty((B, M), np.complex128)
    row_of = np.empty((4, 64, 128), np.int64)    # [j, tg, l] -> row in core
    for tg in range(64):
        G, t4 = tg // 8, tg % 8
        a, h = G // 2, G % 2
        for j in range(4):
            tau = 4 * t4 + j
            cc, t = tau // 8, tau % 8
            blk = a + 4 * (cc % 2)
            colib = 2048 * h + 1024 * (cc // 2) + 128 * t
            row_of[j, tg, :] = 4096 * blk + colib + np.arange(128)
    for n in range(NCORES):
        q = np.asarray(q_cores[n], np.float64).reshape(4, TW, 64, P)  # j, dc, tg, l
        qc = q[:, 0::2] + 1j * q[:, 1::2]                             # j, e, tg, l
        rows = np.empty((RPC, M), np.complex128)
        ro = row_of.reshape(4, 64 * 128)
        for j in range(4):
            rows[ro[j]] = qc[j].reshape(M, 64 * 128).T
        out[n * RPC:(n + 1) * RPC] = rows
    # coefficients: c_0 = 1, c_16 = q_0 - 1, c_d = q_{16-d} for d=1..15
    out *= np.exp(4.0)              # undo the per-group -1 log-mag offset
    q0 = out[:, 0]
    coeffs = np.empty((B, K + 1), np.complex128)
    coeffs[:, 0] = 1.0
    coeffs[:, 16] = q0 - 1.0
    coeffs[:, 1:16] = out[:, 1:][:, ::-1]
    l2 = np.sqrt(np.sum(np.abs(coeffs) ** 2, axis=-1, keepdims=True))
    return coeffs / l2 * np.sqrt(K + 1)


def kernel(x: np.ndarray, shuffle_vector: np.ndarray) -> np.ndarray:
    global _cached
    x = np.asarray(x)
    assert x.shape == (B, K), x.shape
    if _cached is None:
        _cached = _build_module()
    in_maps = _prep_inputs(x, shuffle_vector)
    res = bass_utils.run_bass_kernel_spmd(
        _cached, in_maps, core_ids=list(range(NCORES)))
    return _decode([res.results[n]["q"] for n in range(NCORES)], shuffle_vector)
